# revision 4
# baseline (speedup 1.0000x reference)
# CIN Trainium2 Bass kernel, v2.
#
# Reference (B=512, F0=40, D=32, sizes=[200,200,200]):
#   h0 = x;  per layer: z[b,(i,j),d] = x[b,i,d]*h[b,j,d];  h' = z^T W
#   out = concat(h1,h2,h3, axis=1).sum(-1)  [B, 600]
#
# v2 structure (per core: 64 batches, columns c = b_local*32 + d, C=2048,
# 4 column tiles of 512; within a tile, 4 c-blocks of 128):
#   L0 (classic): h1[n, c] accumulated over 14 chunks of 120 z-rows; z0
#     produced by selector-matmul broadcast + DVE multiply (as baseline).
#   L1 (swapped): out[c_blk, n] = sum_k z1[k, c_blk] * W1[k, n] — every
#     matmul is K=128/72, M=128 (c-block), N=200.  100% PE-cell use, no
#     M=200->128+72 double streaming, and the output lands TRANSPOSED
#     (h2T[c, n]), which is exactly what layer 2 needs.
#   L2 (Gram form): out3 only needs sum_d h3, so
#     GT[j, i, b] = sum_d h2[j,(b,d)] x[b,i,d]   (K=32 matmuls from h2T)
#     out3[n, b]  = sum_{i,j} W2[(i,j),n] GT[j,i,b]   (N=64 GEMM)
#     — no full-width layer-2 z multiply or GEMM at all.
#   out2[b, n] = sum_d h2T[(b4,d), n] via a tiny ones-selector matmul.

import dataclasses

import numpy as np
import ml_dtypes

B, F0, D, S = 512, 40, 32, 200
NCORES = 8
BPC = B // NCORES          # 64 batches per core
C = BPC * D                # 2048 columns per core
CT = 512                   # column tile
NCT = C // CT              # 4 column tiles
NCB = CT // 128            # 4 c-blocks per column tile
L0K = 7                    # layer-0 chunks of 120 folded pair-rows (820+pad)
HI = F0 // 2               # i's per xf half

bf16 = ml_dtypes.bfloat16

_CACHE = {}


def _sub_ap(t, col0, dims):
    """AP at free-offset col0 with explicit free dims [[stride, count], ...]
    (listed slowest-to-fastest)."""
    base = t[:, col0:col0 + 1]
    a = [list(d) for d in base.ap]
    return dataclasses.replace(base, ap=[a[0]] + [list(d) for d in dims])


def _build_nc(repeat=1, skip=(), xf_eng="sp", zb_pool=0, o2f32=0, gt_eng="act",
              gskip=0):
    """xf_eng: which engine queue issues the xf broadcast DMA chains.
    zb_pool: how many of the 20 zb2 multiplies per ct run on GPSIMD.
    o2f32: use fp32 transposes for the out2 path (extra copy, no bf16
    PSUM transpose).  gt_eng: engine for the GT scatter copies.
    gskip: skip the whole Gram/out3 phase (bisect aid)."""
    from contextlib import ExitStack
    import concourse.tile as tile
    from concourse import bacc, mybir

    nsl = slice(0, 4) if "mm" in skip else slice(0, None)

    nc = bacc.Bacc("TRN2", target_bir_lowering=False, debug=False,
                   num_devices=NCORES)

    dt = mybir.dt
    ein, eout = "ExternalInput", "ExternalOutput"
    x0T_d = nc.dram_tensor("x0T", [F0, C], dt.bfloat16, kind=ein).ap()
    w0_d = nc.dram_tensor("w0", [120, L0K * S], dt.bfloat16, kind=ein).ap()
    t0i_d = nc.dram_tensor("t0i", [F0, L0K * 120], dt.bfloat16,
                           kind=ein).ap()
    t0j_d = nc.dram_tensor("t0j", [F0, L0K * 120], dt.bfloat16,
                           kind=ein).ap()
    w1a_d = nc.dram_tensor("w1a", [128, F0 * S], dt.bfloat16, kind=ein).ap()
    w1b_d = nc.dram_tensor("w1b", [72, F0 * S], dt.bfloat16, kind=ein).ap()
    w2a_d = nc.dram_tensor("w2a", [128, F0 * S], dt.bfloat16, kind=ein).ap()
    w2b_d = nc.dram_tensor("w2b", [72, F0 * S], dt.bfloat16, kind=ein).ap()
    F0e = F0 + 1   # extra pseudo-field of ones: its Gram column IS out2^T
    xTd_d = nc.dram_tensor("xTd", [128, (BPC // 4) * 4 * F0e], dt.bfloat16,
                           kind=ein).ap()
    ident_d = nc.dram_tensor("ident", [128, 128], dt.float32, kind=ein).ap()
    y_d = nc.dram_tensor("y", [BPC, 3 * S], dt.float32, kind=eout).ap()

    with tile.TileContext(nc) as tc, ExitStack() as ctx:
        const = ctx.enter_context(tc.tile_pool(name="const", bufs=1))
        xpool = ctx.enter_context(tc.tile_pool(name="xpool", bufs=1))
        zpool = ctx.enter_context(tc.tile_pool(name="zpool", bufs=3))
        hpool = ctx.enter_context(tc.tile_pool(name="hpool", bufs=2))
        ps = ctx.enter_context(tc.tile_pool(name="ps", bufs=2, space="PSUM"))

        def const_tile(name, shape, dtp, src):
            t = const.tile(shape, dtp, name=name, tag=name)
            nc.sync.dma_start(t[:], src[:])
            return t

        x0T = const_tile("x0T_sb", [F0, C], dt.bfloat16, x0T_d)
        w0 = const_tile("w0_sb", [120, L0K * S], dt.bfloat16, w0_d)
        t0i = const_tile("t0i_sb", [F0, L0K * 120], dt.bfloat16, t0i_d)
        t0j = const_tile("t0j_sb", [F0, L0K * 120], dt.bfloat16, t0j_d)
        xTd = const_tile("xTd_sb", [128, (BPC // 4) * 4 * F0e], dt.bfloat16,
                         xTd_d)
        ident = const_tile("ident_sb", [128, 128], dt.float32, ident_d)
        identb = const.tile([128, 128], dt.bfloat16, name="identb",
                            tag="identb")
        nc.vector.tensor_copy(identb[:], ident[:])
        # big weight tiles: loads staggered after first layer-0 chunks
        w1a = const_tile("w1a_sb", [128, F0 * S], dt.bfloat16, w1a_d)
        w1b = const_tile("w1b_sb", [72, F0 * S], dt.bfloat16, w1b_d)
        w2a = const_tile("w2a_sb", [128, F0 * S], dt.bfloat16, w2a_d)
        w2b = const_tile("w2b_sb", [72, F0 * S], dt.bfloat16, w2b_d)

        # layer-0 output (h1) d-sums [n, b_local] fp32
        outA0 = const.tile([128, BPC], dt.float32, name="outA0", tag="outA0")
        outB0 = const.tile([72, BPC], dt.float32, name="outB0", tag="outB0")
        # Gram accumulators in SBUF bf16: GTs[j, i*64 + b]; i=40 is the
        # ones pseudo-field (= out2^T)
        GTsA = const.tile([128, F0e * BPC], dt.bfloat16, name="GTsA",
                          tag="GTsA")
        GTsB = const.tile([72, F0e * BPC], dt.bfloat16, name="GTsB",
                          tag="GTsB")
        final = const.tile([BPC, 3 * S], dt.float32, name="final", tag="final")

        rep_ctx = tc.For_i(0, repeat, 1) if repeat > 1 else None
        if rep_ctx is not None:
            rep_ctx.__enter__()

        zsl = slice(0, 4) if "tt" in skip else slice(0, CT)
        st8 = {}   # per-ct pipeline state

        def emit_xf(ct):
            """x0 broadcast tiles Xf[p, i*CT+c] = x0T[i, ct*CT+c], two halves
            via seed + row-doubling DMA chains (as baseline)."""
            c0 = ct * CT
            csl = slice(c0, c0 + CT)
            eng = {"sp": nc.sync, "pool": nc.gpsimd, "act": nc.scalar,
                   "dve": nc.vector}[xf_eng]
            xfs = []
            for h in range(2):
                xf = xpool.tile([128, HI * CT], dt.bfloat16,
                                name=f"xf{h}_{ct}", tag="xfh", bufs=3)
                if "xf" in skip:
                    eng.dma_start(
                        xf[0:1, 0:HI].rearrange("p (i c) -> p i c", c=1),
                        x0T_d[h * HI:(h + 1) * HI, c0:c0 + 1])
                else:
                    for r in range(8):
                        eng.dma_start(
                            xf[r:r + 1, :].rearrange("p (i c) -> p i c", c=CT),
                            x0T_d[h * HI:(h + 1) * HI, csl])
                    n = 8
                    while n < 128:
                        eng.dma_start(xf[n:2 * n, :], xf[0:n, :])
                        n *= 2
                xfs.append(xf)
            st8[ct] = {"xfs": xfs}

        def xsl4(ct, i, rows=slice(0, 128)):
            """xf slice covering i..i+4 (same half; HI=20 % 4 == 0)."""
            h, ii = divmod(i, HI)
            return st8[ct]["xfs"][h][rows, ii * CT:(ii + 4) * CT]

        def l0_feed(ct, k):
            """Folded layer-0 chunk k: two selector-broadcast patterns
            (x_i and x_j of the pair list) multiplied into z0."""
            c0 = ct * CT
            csl = slice(c0, c0 + CT)
            pats = []
            for nm, sel in (("i", t0i), ("j", t0j)):
                xp = ps.tile([128, CT], dt.float32, name=f"xp{nm}{k}",
                             tag="tps", bufs=1)
                nc.tensor.matmul(xp[0:120, :],
                                 sel[:, k * 120:(k + 1) * 120],
                                 x0T[:, csl], start=True, stop=True)
                x0p = xpool.tile([120, CT], dt.bfloat16, name=f"x0p{nm}{k}",
                                 tag=f"x0p{nm}", bufs=3)
                nc.scalar.copy(x0p[:], xp[0:120, :])
                pats.append(x0p)
            z0 = zpool.tile([120, CT], dt.bfloat16, name=f"z0_{ct}_{k}",
                            tag="z0", bufs=4)
            nc.vector.tensor_mul(z0[:, zsl], pats[0][:, zsl], pats[1][:, zsl])
            st8[ct].setdefault("z0s", {})[k] = z0

        def emit_l0(ct, first=False):
            """Layer 0 GEMM (classic [n, c] orientation)."""
            accA = ps.tile([128, CT], dt.float32, name=f"l0A{ct}", tag="accA",
                           bufs=1)
            accB = ps.tile([72, CT], dt.float32, name=f"l0B{ct}", tag="accB",
                           bufs=1)
            z0s = st8[ct]["z0s"]
            for k in range(L0K):
                if k not in z0s:
                    l0_feed(ct, k)
                z0 = z0s[k]
                nc.tensor.matmul(accA[:, nsl], w0[:, k * S:k * S + 128],
                                 z0[:, nsl],
                                 start=(k == 0), stop=(k == L0K - 1))
                nc.tensor.matmul(accB[:, nsl], w0[:, k * S + 128:(k + 1) * S],
                                 z0[:, nsl],
                                 start=(k == 0), stop=(k == L0K - 1))
            st8[ct]["acc"] = (accA, accB)

        def bcast4(ap):
            a = [list(d) for d in ap.ap]
            return dataclasses.replace(ap, ap=[a[0], [0, 4], a[1]])

        def drain_l0(ct):
            """h1 bf16 tiles + deferred out1 d-sum reduces."""
            accA, accB = st8[ct]["acc"]
            hA = hpool.tile([128, CT], dt.bfloat16, name=f"hA{ct}", tag="hA")
            hB = hpool.tile([72, CT], dt.bfloat16, name=f"hB{ct}", tag="hB")
            nc.scalar.copy(hA[:], accA[:])
            nc.scalar.copy(hB[:], accB[:])

            from concourse import mybir as mb

            def reduces():
                nc.vector.tensor_reduce(
                    outA0[:, ct * 16:(ct + 1) * 16],
                    accA[:].rearrange("p (b d) -> p b d", d=D),
                    axis=mb.AxisListType.X, op=mb.AluOpType.add)
                nc.vector.tensor_reduce(
                    outB0[:, ct * 16:(ct + 1) * 16],
                    accB[:].rearrange("p (b d) -> p b d", d=D),
                    axis=mb.AxisListType.X, op=mb.AluOpType.add)

            return hA, hB, reduces

        def emit_l1(ct, hA, hB, feeds=None):
            """Swapped layer-1: accT[c_blk, n] += z[k, c_blk]^T W1[k, n].
            Two [128, 2*S] psum tiles hold the 4 c-block accumulators."""
            accts = [ps.tile([128, CT], dt.float32, name=f"l1T{ct}_{q}",
                             tag="accT", bufs=4) for q in range(NCB)]

            def acct(cb):
                return accts[cb][:, 0:S]

            for j0 in range(0, F0, 4):
                x4 = xsl4(ct, j0)
                za4 = zpool.tile([128, 4 * CT], dt.bfloat16,
                                 name=f"za{ct}_{j0}", tag="za", bufs=2)
                nc.vector.tensor_mul(
                    za4[:].rearrange("p (r c) -> p r c", r=4),
                    bcast4(hA[:]),
                    x4.rearrange("p (r c) -> p r c", r=4))
                zb4 = zpool.tile([72, 4 * CT], dt.bfloat16,
                                 name=f"zb{ct}_{j0}", tag="zb", bufs=2)
                zb_eng = nc.gpsimd if (j0 // 4) < zb_pool else nc.vector
                zb_eng.tensor_mul(
                    zb4[:].rearrange("p (r c) -> p r c", r=4),
                    bcast4(hB[:]),
                    xsl4(ct, j0, slice(0, 72)).rearrange(
                        "p (r c) -> p r c", r=4))
                for di in range(4):
                    i = j0 + di
                    for cb in range(NCB):
                        lA = za4[:, di * CT + cb * 128:di * CT + (cb + 1) * 128]
                        lB = zb4[:, di * CT + cb * 128:di * CT + (cb + 1) * 128]
                        nc.tensor.matmul(acct(cb)[:, nsl],
                                         lA, w1a[:, i * S:(i + 1) * S][:, nsl],
                                         start=(i == 0), stop=False)
                        nc.tensor.matmul(acct(cb)[:, nsl],
                                         lB, w1b[:, i * S:(i + 1) * S][:, nsl],
                                         start=False, stop=(i == F0 - 1))
                if feeds:
                    if feeds:
                        feeds.pop(0)()
            while feeds:
                feeds.pop(0)()
            st8[ct]["accT"] = acct

        def drain_l1(ct):
            """h2T bf16 [c, (cb, n)] per c-block (interleaved with emit_g)."""
            acct = st8[ct]["accT"]
            h2T = hpool.tile([128, NCB * S], dt.bfloat16, name=f"h2T{ct}",
                             tag="h2T", bufs=2)
            st8[ct]["h2T"] = h2T

        def emit_g(ct):
            """Per c-block: drain copy then its Gram matmuls, so G(cb) runs
            on the PE while copy(cb+1) is still on ACT.
            GT[j, i, b] = sum_d h2T[(b4,d), cb*S+j] * xTd[(b4,d), g*40+i]"""
            h2T = st8[ct]["h2T"]
            acct = st8[ct]["accT"]
            for cb in range(NCB):
                nc.scalar.copy(h2T[:, cb * S:(cb + 1) * S], acct(cb))
                g = ct * NCB + cb
                gmix = ps.tile([128, 512], dt.float32, name=f"gt{ct}_{cb}",
                               tag="gmix", bufs=1)
                gta = gmix[:, 0:4 * F0e]
                gtb = gmix[0:72, 4 * F0e:8 * F0e]
                for b4 in range(4):
                    # K=128 at base 0; xTd zeros out the other b4s' rows
                    gi = g * 4 + b4
                    rhs = xTd[:, gi * F0e:(gi + 1) * F0e]
                    nc.tensor.matmul(gta[:, b4 * F0e:(b4 + 1) * F0e],
                                     h2T[:, cb * S:cb * S + 128],
                                     rhs, start=True, stop=True)
                    nc.tensor.matmul(gtb[:, b4 * F0e:(b4 + 1) * F0e],
                                     h2T[:, cb * S + 128:(cb + 1) * S],
                                     rhs, start=True, stop=True)
                # scatter [j, (b4, i)] -> GTs[j, i*64 + (g*4 + b4)]
                b0 = g * 4
                dims_out = [[BPC, F0e], [1, 4]]
                dims_in = [[1, F0e], [F0e, 4]]
                geng = nc.scalar if gt_eng == "act" else nc.vector
                cp = geng.copy if gt_eng == "act" else geng.tensor_copy
                cp(_sub_ap(GTsA, b0, dims_out), _sub_ap(gta, 0, dims_in))
                cp(_sub_ap(GTsB, b0, dims_out), _sub_ap(gtb, 0, dims_in))

        def transpose_to_final(cols, dtp, idn, tA, tB):
            tpA = ps.tile([BPC, 128], dtp, name=f"tpA{cols}",
                          tag="tps", bufs=1)
            nc.tensor.transpose(tpA[:], tA, idn[:])
            nc.scalar.copy(final[:, cols:cols + 128], tpA[:])
            tpB = ps.tile([BPC, 72], dtp, name=f"tpB{cols}",
                          tag="tps", bufs=1)
            nc.tensor.transpose(tpB[:], tB, idn[0:72, 0:72])
            nc.scalar.copy(final[:, cols + 128:cols + 200], tpB[:])

        def final_out():
            # out1/out2 transposes don't depend on the out3 GEMM: emit them
            # first so their ACT copies overlap it and only out3's transpose
            # sits on the iteration tail
            transpose_to_final(0, dt.float32, ident, outA0[:], outB0[:])
            o2A = GTsA[:, F0 * BPC:(F0 + 1) * BPC]
            o2B = GTsB[:, F0 * BPC:(F0 + 1) * BPC]
            if o2f32:
                o2Af = hpool.tile([128, BPC], dt.float32, name="o2Af",
                                  tag="o2Af", bufs=1)
                o2Bf = hpool.tile([72, BPC], dt.float32, name="o2Bf",
                                  tag="o2Bf", bufs=1)
                nc.vector.tensor_copy(o2Af[:], o2A)
                nc.vector.tensor_copy(o2Bf[:], o2B)
                transpose_to_final(200, dt.float32, ident, o2Af[:], o2Bf[:])
            else:
                transpose_to_final(200, dt.bfloat16, identb, o2A, o2B)

            # out3[n, b] = sum_{i, j} W2[(i,j), n] GT[j, i*64+b]
            o3A = ps.tile([128, BPC], dt.float32, name="o3A", tag="accA",
                          bufs=1)
            o3B = ps.tile([72, BPC], dt.float32, name="o3B", tag="accB",
                          bufs=1)
            for i in range(F0):
                st = (i == 0)
                sp = (i == F0 - 1)
                nc.tensor.matmul(o3A[:], w2a[:, i * S:i * S + 128],
                                 GTsA[:, i * BPC:(i + 1) * BPC],
                                 start=st, stop=False)
                nc.tensor.matmul(o3A[:], w2b[:, i * S:i * S + 128],
                                 GTsB[:, i * BPC:(i + 1) * BPC],
                                 start=False, stop=sp)
                nc.tensor.matmul(o3B[:], w2a[:, i * S + 128:(i + 1) * S],
                                 GTsA[:, i * BPC:(i + 1) * BPC],
                                 start=st, stop=False)
                nc.tensor.matmul(o3B[:], w2b[:, i * S + 128:(i + 1) * S],
                                 GTsB[:, i * BPC:(i + 1) * BPC],
                                 start=False, stop=sp)
            o3As = hpool.tile([128, BPC], dt.float32, name="o3As", tag="o3As",
                              bufs=1)
            o3Bs = hpool.tile([72, BPC], dt.float32, name="o3Bs", tag="o3Bs",
                              bufs=1)
            nc.vector.tensor_copy(o3As[:], o3A[:])
            nc.vector.tensor_copy(o3Bs[:], o3B[:])
            transpose_to_final(400, dt.float32, ident, o3As[:], o3Bs[:])

        # ---- software-pipelined emission over column tiles ----
        emit_xf(0)
        for k in range(L0K):
            l0_feed(0, k)
        emit_l0(0, first=True)
        for ct in range(NCT):
            hA, hB, red0 = drain_l0(ct)
            if ct + 1 < NCT:
                emit_xf(ct + 1)
                feeds = [red0] + \
                    [(lambda cc, kk: lambda: l0_feed(cc, kk))(ct + 1, k)
                     for k in range(L0K)]
            else:
                feeds = [red0]
            emit_l1(ct, hA, hB, feeds=feeds)
            if ct + 1 < NCT:
                emit_l0(ct + 1)
            drain_l1(ct)
            if not gskip:
                emit_g(ct)
            st8.pop(ct - 1, None)
        if not gskip:
            final_out()
        else:
            for cols, (tA, tB) in ((0, (outA0, outB0)),):
                tpA = ps.tile([BPC, 128], dt.float32, name=f"tpA{cols}",
                              tag="tps", bufs=1)
                nc.tensor.transpose(tpA[:], tA[:], ident[:])
                nc.scalar.copy(final[:, cols:cols + 128], tpA[:])
                tpB = ps.tile([BPC, 72], dt.float32, name=f"tpB{cols}",
                              tag="tps", bufs=1)
                nc.tensor.transpose(tpB[:], tB[:], ident[0:72, 0:72])
                nc.scalar.copy(final[:, cols + 128:cols + 200], tpB[:])

        # y goes out on the ACT queue: on SP it would head-block the next
        # iteration's xf broadcast chains behind the final-copy semaphores
        nc.scalar.dma_start(y_d[:], final[:])

        if rep_ctx is not None:
            rep_ctx.__exit__(None, None, None)

    nc.compile()
    return nc


def _prep_consts(W0, W1, W2):
    """Host-side constant tensors shared by all cores (bf16)."""
    # layer 0: z[b,(i,j),d] is symmetric in (i,j), so fold
    # W0' [(i,j) i<=j] = W0[(i,j)] + W0[(j,i)] (diag once): 1600 -> 820 rows
    pairs = [(i, j) for i in range(F0) for j in range(i, F0)]
    W0f = np.zeros((L0K * 120, S), dtype=np.float32)
    for r, (i, j) in enumerate(pairs):
        W0f[r] = W0[i * F0 + j] + (W0[j * F0 + i] if i != j else 0)
    w0p = np.zeros((120, L0K * S), dtype=bf16)
    for k in range(L0K):
        w0p[:, k * S:(k + 1) * S] = W0f[k * 120:(k + 1) * 120].astype(bf16)

    # one-hot selectors broadcasting x0T rows i(p), j(p) per chunk row
    t0i = np.zeros((F0, L0K * 120), dtype=bf16)
    t0j = np.zeros((F0, L0K * 120), dtype=bf16)
    for r, (i, j) in enumerate(pairs):
        t0i[i, r] = 1.0
        t0j[j, r] = 1.0

    # layers 1/2 weights, j-major: w[j, i*S + n] = W[i*S + j, n]
    def wperm(W):
        P = np.ascontiguousarray(
            W.reshape(F0, S, S).transpose(1, 0, 2).reshape(S, F0 * S)
        ).astype(bf16)
        return P[0:128], P[128:S]

    w1a, w1b = wperm(np.asarray(W1, np.float32))
    w2a, w2b = wperm(np.asarray(W2, np.float32))

    ident = np.eye(128, dtype=np.float32)
    return dict(w0=w0p, w1a=w1a, w1b=w1b, w2a=w2a, w2b=w2b, ident=ident,
                t0i=t0i, t0j=t0j)


def _prep_in_maps(inputs, W0, W1, W2):
    consts = _prep_consts(np.asarray(W0, np.float32),
                          np.asarray(W1, np.float32),
                          np.asarray(W2, np.float32))
    x = np.asarray(inputs, np.float32)
    in_maps = []
    for c in range(NCORES):
        xs = x[c * BPC:(c + 1) * BPC]                    # [64, 40, 32]
        x0T = np.ascontiguousarray(
            xs.transpose(1, 0, 2).reshape(F0, C)).astype(bf16)
        # xTd[(b4, d), b_loc*F0e + i] = x[b, i, d] on b's own 32 d-rows,
        # zero elsewhere, so a base-0 K=128 matmul contracts only b's rows.
        # Column i=F0 is all-ones (same mask): its Gram output is
        # sum_d h2[j, (b, d)] = out2^T.
        F0e = F0 + 1
        xTd = np.zeros((128, BPC * F0e), dtype=bf16)
        for b_loc in range(BPC):
            b4 = b_loc % 4
            c0 = b_loc * F0e
            xTd[b4 * 32:(b4 + 1) * 32, c0:c0 + F0] = xs[b_loc].T.astype(bf16)
            xTd[b4 * 32:(b4 + 1) * 32, c0 + F0] = 1.0
        in_maps.append(dict(x0T=x0T, xTd=xTd, **consts))
    return in_maps


def _get_nc():
    if "nc" not in _CACHE:
        _CACHE["nc"] = _build_nc()
    return _CACHE["nc"]


def kernel(inputs, W0, W1, W2):
    from concourse.bass_utils import run_bass_kernel_spmd

    nc = _get_nc()
    in_maps = _prep_in_maps(inputs, W0, W1, W2)
    res = run_bass_kernel_spmd(nc, in_maps, core_ids=list(range(NCORES)))
    y = np.concatenate([res.results[c]["y"] for c in range(NCORES)], axis=0)
    return np.ascontiguousarray(y, dtype=np.float32)
